# revision 10
# baseline (speedup 1.0000x reference)
import numpy as np, sys, os, math, functools
sys.path.insert(0, "/opt/trn_rl_repo")

V, D, L = 32000, 1024, 8
HQ, HKV, HD = 16, 4, 64
H = 2752
B, S = 2, 1024
WINDOW, GEVERY = 256, 4
EPS, BASE = 1e-6, 10000.0
NCORES = 8
T = 256
VSH = V // NCORES
NEG = -30000.0
SCALE = 1.0 / 8.0
SIM_SILU = False  # CoreSim lacks Silu; emulate with Sigmoid+mul when set
SKIP_COLL = False  # timing experiment: drop collectives
SKIP_LM = False    # timing experiment: drop lm-head compute
SKIP_ATTN = False  # timing experiment: drop attention score/AV work

# slot g holds query head HPERM[g]; chosen so (g%2) == (HPERM[g]//4)%2,
# i.e. each head's partition parity matches its kv head's parity in kT.
HPERM = [0, 4, 1, 5, 2, 6, 3, 7, 8, 12, 9, 13, 10, 14, 11, 15]
KVH_OF_SLOT = [HPERM[g] // 4 for g in range(16)]
KVGROUPS = [[g for g in range(16) if KVH_OF_SLOT[g] == kvh] for kvh in range(4)]
# column block j of kT_full covers position block POSBLK[j]
POSBLK = [0, 1, 2, 3, 7, 6, 5, 4]
# w1/w3 DMA chunks and the derived 128-row hchunks
W13CH = [(0, 512), (512, 512), (1024, 512), (1536, 512), (2048, 512), (2560, 192)]
HCHUNKS = [(i * 128, 128) for i in range(21)] + [(2688, 64)]

def _core_blocks(c):
    cp = c % 4
    return cp, 7 - cp

def _inv_freq():
    return 1.0 / (BASE ** (np.arange(0, HD, 2, dtype=np.float64) / HD))

def _host_masks(c):
    bA, bB = _core_blocks(c)
    p = np.arange(128)
    qA = bA * 128 + p
    qB = bB * 128 + p
    posblk = np.asarray(POSBLK)
    def mk(qpos, c0, w, local):
        cols = c0 + np.arange(w)
        kpos = posblk[cols // 128] * 128 + cols % 128
        valid = kpos[None, :] <= qpos[:, None]
        if local:
            valid &= (qpos[:, None] - kpos[None, :]) < WINDOW
        return np.where(valid, 0.0, NEG).astype(np.float16)
    return (mk(qA, 0, 512, False), mk(qB, 512, 512, False),
            mk(qA, 0, 512, True), mk(qB, 256, 768, True))

def _host_prep(idx, emb, Wq, Wk, Wv, Wo, w1, w3, w2, n1, n2, nf):
    idx = np.asarray(idx); emb = np.asarray(emb, dtype=np.float32)
    invf = _inv_freq()
    # permute q heads (wq cols) and wo rows by HPERM
    Wq = np.asarray(Wq, dtype=np.float16).reshape(L, D, HQ, HD)
    Wq = np.ascontiguousarray(Wq[:, :, HPERM, :].reshape(L, D, HQ * HD))
    Wo = np.asarray(Wo, dtype=np.float16).reshape(L, HQ, HD, D)
    Wo = np.ascontiguousarray(Wo[:, HPERM, :, :].reshape(L, HQ * HD, D))
    wk16 = np.asarray(Wk, dtype=np.float16)
    wv16 = np.asarray(Wv, dtype=np.float16)
    w116 = np.asarray(w1, dtype=np.float16)
    w316 = np.asarray(w3, dtype=np.float16)
    w216 = np.asarray(w2, dtype=np.float16)
    n1h = np.ascontiguousarray(np.asarray(n1).reshape(L, 8, 128).transpose(0, 2, 1))
    n2h = np.ascontiguousarray(np.asarray(n2).reshape(L, 8, 128).transpose(0, 2, 1))
    nfh = np.ascontiguousarray(np.asarray(nf).reshape(8, 128).T)
    in_maps = []
    for c in range(NCORES):
        s = c // 4
        bA, bB = _core_blocks(c)
        tok = np.concatenate([idx[s, bA*128:(bA+1)*128], idx[s, bB*128:(bB+1)*128]])
        x0T = np.ascontiguousarray(emb[tok].T)
        pos = np.concatenate([bA*128 + np.arange(128), bB*128 + np.arange(128)])
        ang = pos[:, None].astype(np.float64) * invf[None, :]
        cosq = np.cos(ang).astype(np.float32).reshape(2, 128, 32)
        sinq = np.sin(ang).astype(np.float32).reshape(2, 128, 32)
        mAg, mBg, mAl, mBl = _host_masks(c)
        embT = np.ascontiguousarray(emb[c*VSH:(c+1)*VSH].T.astype(np.float16))
        in_maps.append({
            "x0T": x0T,
            "wq": Wq, "wk": wk16, "wv": wv16, "wo": Wo,
            "w1": w116, "w3": w316, "w2": w216,
            "n1h": n1h, "n2h": n2h, "nfh": nfh,
            "cosq": cosq, "sinq": sinq,
            "mAg": mAg, "mBg": mBg, "mAl": mAl, "mBl": mBl,
            "embT": embT,
        })
    return in_maps

def _unperm_rows():
    perm = np.zeros(2048, dtype=np.int64)
    for r in range(2048):
        rr, rem = divmod(r, 256)
        slot, p = divmod(rem, 128)
        samp = rr // 4
        bA, bB = _core_blocks(rr)
        blk = bA if slot == 0 else bB
        perm[r] = samp * S + blk * 128 + p
    inv = np.zeros(2048, dtype=np.int64)
    inv[perm] = np.arange(2048)
    return inv

def _assemble(outs):
    fullT = np.empty((V, 2048), dtype=np.float32)
    for c in range(NCORES):
        fullT[c*VSH:(c+1)*VSH, :] = outs[c]["logits"]
    full = np.ascontiguousarray(fullT.T)
    inv = _unperm_rows()
    return full[inv].reshape(B, S, V)

def _build_nc():
    import concourse.bass as bass
    import concourse.bacc as bacc
    import concourse.mybir as mybir
    from concourse.tile import TileContext
    from concourse.masks import make_identity
    F32, F16 = mybir.dt.float32, mybir.dt.float16
    AF = mybir.ActivationFunctionType
    ALU = mybir.AluOpType

    nc = bacc.Bacc("TRN2", target_bir_lowering=False, debug=False, num_devices=NCORES)
    P = {}
    def inp(name, shape, dt=F32):
        P[name] = nc.declare_dram_parameter(name, list(shape), dt, isOutput=False)
    inp("x0T", (D, T))
    inp("wq", (L, D, D), F16); inp("wk", (L, D, 256), F16); inp("wv", (L, D, 256), F16)
    inp("wo", (L, D, D), F16)
    inp("w1", (L, D, H), F16); inp("w3", (L, D, H), F16); inp("w2", (L, H, D), F16)
    inp("n1h", (L, 128, 8)); inp("n2h", (L, 128, 8)); inp("nfh", (128, 8))
    inp("cosq", (2, 128, 32)); inp("sinq", (2, 128, 32))
    inp("mAg", (128, 512), F16); inp("mBg", (128, 512), F16)
    inp("mAl", (128, 512), F16); inp("mBl", (128, 768), F16)
    inp("embT", (D, VSH), F16)
    logits = nc.declare_dram_parameter("logits", [VSH, 2048], F16, isOutput=True)

    kv_in  = [nc.dram_tensor(f"kv_in{l}",  [4, 128, 256], F16) for l in range(L)]
    kv_out = [nc.dram_tensor(f"kv_out{l}", [16, 128, 256], F16) for l in range(L)]
    xf_in  = nc.dram_tensor("xf_in", [D, T], F16)
    xf_out = nc.dram_tensor("xf_out", [NCORES * D, T], F16, addr_space="Shared")
    RG_KV = [[0, 1, 2, 3], [4, 5, 6, 7]]
    RG_ALL = [list(range(NCORES))]

    with TileContext(nc) as tc:
      with tc.tile_pool(name="pers", bufs=1) as pers, \
           tc.tile_pool(name="wpool", bufs=1) as wp, \
           tc.tile_pool(name="act", bufs=1) as act, \
           tc.tile_pool(name="attn", bufs=1) as atp, \
           tc.tile_pool(name="small", bufs=4) as sm, \
           tc.tile_pool(name="ppb", bufs=2, space="PSUM") as ppb, \
           tc.tile_pool(name="ppo", bufs=2, space="PSUM") as ppo, \
           tc.tile_pool(name="ppm", bufs=3, space="PSUM") as ppm, \
           tc.tile_pool(name="ppt", bufs=1, space="PSUM") as ppt:

        dma = nc.sync.dma_start
        xT = [pers.tile([128, T], F32, tag=f"xT{d}", name=f"xT{d}") for d in range(8)]
        for d in range(8):
            dma(out=xT[d], in_=P["x0T"][d*128:(d+1)*128, :])
        cosA = pers.tile([128, 32], F32, tag="cosA", name="cosA"); dma(out=cosA, in_=P["cosq"][0])
        cosB = pers.tile([128, 32], F32, tag="cosB", name="cosB"); dma(out=cosB, in_=P["cosq"][1])
        sinA = pers.tile([128, 32], F32, tag="sinA", name="sinA"); dma(out=sinA, in_=P["sinq"][0])
        sinB = pers.tile([128, 32], F32, tag="sinB", name="sinB"); dma(out=sinB, in_=P["sinq"][1])
        mAg = pers.tile([128, 512], F16, tag="mAg", name="mAg"); dma(out=mAg, in_=P["mAg"][:, :])
        mBg = pers.tile([128, 512], F16, tag="mBg", name="mBg"); dma(out=mBg, in_=P["mBg"][:, :])
        mAl = pers.tile([128, 512], F16, tag="mAl", name="mAl"); dma(out=mAl, in_=P["mAl"][:, :])
        mBl = pers.tile([128, 768], F16, tag="mBl", name="mBl"); dma(out=mBl, in_=P["mBl"][:, :])
        idn = pers.tile([128, 128], F16, tag="idn", name="idn")
        make_identity(nc, idn)
        ones16 = pers.tile([128, 1], F16, tag="ones16", name="ones16")
        nc.vector.memset(ones16, 1.0)
        epst = pers.tile([1, 1], F32, tag="epst", name="epst")
        nc.vector.memset(epst, EPS)

        def rmsnorm(nw_dram):
            nw = sm.tile([128, 8], F32, tag="nw", name="nw")
            dma(out=nw, in_=nw_dram)
            ssp = ppm.tile([128, T], F32, tag="pm", name="ssp")
            for d in range(8):
                x2 = act.tile([128, T], F16, tag="x2", name="x2", bufs=2)
                nc.vector.tensor_mul(out=x2, in0=xT[d], in1=xT[d])
                nc.tensor.matmul(ssp[0:1, :], lhsT=ones16, rhs=x2, start=(d == 0), stop=(d == 7))
            lnr = sm.tile([1, T], F32, tag="lnr", name="lnr")
            nc.scalar.activation(out=lnr, in_=ssp[0:1, :], func=AF.Ln, scale=1.0/D,
                                 bias=epst[0:1, 0:1])
            rr = sm.tile([1, T], F32, tag="rr", name="rr")
            nc.scalar.activation(out=rr, in_=lnr, func=AF.Exp, scale=-0.5)
            rb = act.tile([128, T], F32, tag="rb", name="rb")
            nc.gpsimd.partition_broadcast(rb[:], rr[:])
            out = []
            for d in range(8):
                h = act.tile([128, T], F16, tag=f"hT{d}", name=f"hT{d}", bufs=1)
                nc.vector.scalar_tensor_tensor(out=h, in0=xT[d], scalar=nw[:, d:d+1],
                                               in1=rb, op0=ALU.mult, op1=ALU.mult)
                out.append(h)
            return out

        def rope_tok(ps, cost, sint, outt, nheads):
            ev = ps.rearrange("p (h f two) -> p h f two", two=2, f=32)
            ov = outt.rearrange("p (h f two) -> p h f two", two=2, f=32)
            cb = cost[:].rearrange("p (o f) -> p o f", o=1).to_broadcast((128, nheads, 32))
            sb = sint[:].rearrange("p (o f) -> p o f", o=1).to_broadcast((128, nheads, 32))
            t1 = sm.tile([128, nheads, 32], F32, tag="ropet1", name="ropet1", bufs=2)
            t2 = sm.tile([128, nheads, 32], F32, tag="ropet2", name="ropet2", bufs=2)
            nc.vector.tensor_mul(out=t1, in0=ev[:, :, :, 0], in1=cb)
            nc.vector.tensor_mul(out=t2, in0=ev[:, :, :, 1], in1=sb)
            nc.vector.tensor_sub(out=ov[:, :, :, 0], in0=t1, in1=t2)
            nc.vector.tensor_mul(out=t1, in0=ev[:, :, :, 0], in1=sb)
            nc.vector.tensor_mul(out=t2, in0=ev[:, :, :, 1], in1=cb)
            nc.vector.tensor_add(out=ov[:, :, :, 1], in0=t1, in1=t2)

        for l in range(L):
            is_global = ((l + 1) % GEVERY) == 0
            h1 = rmsnorm(P["n1h"][l])
            # ---- QKV projections (shared stationary h1 slices) ----
            wqt = wp.tile([128, 8, 1024], F16, tag="wbig", name="wqt", bufs=2)
            dma(out=wqt, in_=P["wq"][l].rearrange("(dc p) f -> p dc f", p=128))
            wkt = wp.tile([128, 8, 256], F16, tag="wkt", name="wkt")
            wvt = wp.tile([128, 8, 256], F16, tag="wvt", name="wvt")
            dma(out=wkt, in_=P["wk"][l].rearrange("(dc p) f -> p dc f", p=128))
            dma(out=wvt, in_=P["wv"][l].rearrange("(dc p) f -> p dc f", p=128))
            ktok = act.tile([128, 512], F16, tag="ktok", name="ktok")
            vtok = act.tile([128, 512], F16, tag="vtok", name="vtok")
            qtoks = [act.tile([128, 1024], F16, tag=f"qtok{t}", name=f"qtok{t}", bufs=1) for t in range(2)]
            for t2_ in range(2):
                psk = ppm.tile([128, 256], F32, tag="pm", name="psk")
                psv = ppm.tile([128, 256], F32, tag="pm", name="psv")
                psq0 = ppb.tile([128, 512], F32, tag="pb", name="psq0")
                psq1 = ppb.tile([128, 512], F32, tag="pb", name="psq1")
                for d in range(8):
                    lh = h1[d][:, t2_*128:(t2_+1)*128]
                    st, sp = (d == 0), (d == 7)
                    nc.tensor.matmul(psk, lhsT=lh, rhs=wkt[:, d, :], start=st, stop=sp)
                    nc.tensor.matmul(psv, lhsT=lh, rhs=wvt[:, d, :], start=st, stop=sp)
                    nc.tensor.matmul(psq0, lhsT=lh, rhs=wqt[:, d, 0:512], start=st, stop=sp)
                    nc.tensor.matmul(psq1, lhsT=lh, rhs=wqt[:, d, 512:1024], start=st, stop=sp)
                rope_tok(psk, (cosA, cosB)[t2_], (sinA, sinB)[t2_], ktok[:, t2_*256:(t2_+1)*256], 4)
                nc.scalar.activation(out=vtok[:, t2_*256:(t2_+1)*256], in_=psv, func=AF.Copy)
                rope_tok(psq0, (cosA, cosB)[t2_], (sinA, sinB)[t2_], qtoks[t2_][:, 0:512], 8)
                rope_tok(psq1, (cosA, cosB)[t2_], (sinA, sinB)[t2_], qtoks[t2_][:, 512:1024], 8)
            # ---- transpose k (4 blocks) into kv_in layout ----
            kTsb = act.tile([128, 512], F16, tag="kTsb", name="kTsb")
            pstk = ppt.tile([128, 1024], F16, tag="pstr", name="pstk")
            for i in range(2):
                for t2_ in range(2):
                    nc.tensor.transpose(pstk[:, i*256 + t2_*128:i*256 + (t2_+1)*128],
                                        ktok[:, t2_*256 + i*128:t2_*256 + (i+1)*128], idn)
            nc.scalar.activation(out=kTsb, in_=pstk[:, 0:512], func=AF.Copy)
            for i in range(2):
                dma(out=kv_in[l][i], in_=kTsb[:, i*256:(i+1)*256])
                dma(out=kv_in[l][2 + i], in_=vtok[:, i*256:(i+1)*256])
            if not SKIP_COLL:
                nc.gpsimd.collective_compute(
                    "AllGather", mybir.AluOpType.bypass, replica_groups=RG_KV,
                    ins=[kv_in[l].ap()], outs=[kv_out[l].ap()])
            # ---- transpose q (16 blocks) -> qT [128, 8, 256] ----
            qT = atp.tile([128, 8, 256], F16, tag="qT", name="qT")
            for t2_ in range(2):
                pst = ppt.tile([128, 1024], F16, tag="pstr", name="pst")
                for f in range(8):
                    nc.tensor.transpose(pst[:, f*128:(f+1)*128],
                                        qtoks[t2_][:, f*128:(f+1)*128], idn)
                dstv = qT[:, :, t2_*128:(t2_+1)*128]
                srcv = pst.rearrange("p (f c) -> p f c", f=8)
                if t2_ == 0:
                    nc.vector.tensor_copy(out=dstv, in_=srcv)
                else:
                    nc.scalar.activation(out=dstv, in_=srcv, func=AF.Copy)
            # ---- gather K/V from collective ----
            kT_full = [atp.tile([128, 1024], F16, tag=f"kTf{i}", name=f"kTf{i}", bufs=1) for i in range(2)]
            kvo = kv_out[l].rearrange("(r x) p c -> x r p c", x=4)
            for i in range(2):
                dma(out=kT_full[i].rearrange("p (sl r c) -> p sl r c", sl=2, r=4),
                    in_=kvo[i].rearrange("r p (sl c) -> p sl r c", sl=2))
            v_full = atp.tile([128, 8, 4, 64], F16, tag="vfull", name="vfull")
            for sl in range(2):
                dma(out=v_full[:, sl*4:(sl+1)*4, :, :],
                    in_=kvo[2 + sl].rearrange("r p (h f) -> p r h f", f=64))
            # ---- attention ----
            oT = atp.tile([128, 8, 256], F16, tag="oT", name="oT")
            for qb in range(2):
                if qb == 0:
                    chunks = [(0, 512, mAg if is_global else mAl)]
                    kblocks = [0, 1, 2, 3]
                elif is_global:
                    chunks = [(0, 512, None), (512, 512, mBg)]
                    kblocks = list(range(8))
                else:
                    chunks = [(256, 512, mBl[:, 0:512]), (768, 256, mBl[:, 512:768])]
                    kblocks = [2, 3, 4, 5, 6, 7]
                kn = sum(w for _, w, _ in chunks)
                nblk = len(kblocks)
                ops = [ppo.tile([128, 512], F32, tag=f"ops{t}", name=f"ops{t}", bufs=1) for t in range(2)]
                for kvh in range(4 if not SKIP_ATTN else 0):
                    ktile = kT_full[kvh // 2]
                    for half in range(2):
                        slots = KVGROUPS[kvh][half*2:(half+1)*2]
                        attTs = []
                        for si, g in enumerate(slots):
                            base = (g % 2) * 64
                            lhq = qT[base:base+64, g//2, qb*128:(qb+1)*128]
                            probs = atp.tile([128, 1024], F16, tag=f"probs{si}", name=f"probs{si}", bufs=1)
                            spss = []
                            for (c0, w, msk) in chunks:
                                sps = ppb.tile([128, 512], F32, tag="pb", name="sps")
                                nc.tensor.matmul(sps[:, 0:w], lhsT=lhq,
                                                 rhs=ktile[base:base+64, c0:c0+w],
                                                 start=True, stop=(msk is None))
                                spss.append(sps)
                            for sps, (c0, w, msk) in zip(spss, chunks):
                                if msk is not None:
                                    nc.tensor.matmul(sps[:, 0:w], lhsT=idn, rhs=msk,
                                                     start=False, stop=True)
                            off = 0
                            accs = []
                            for sps, (c0, w, msk) in zip(spss, chunks):
                                acc = sm.tile([128, 1], F32, tag="acc", name="acc")
                                nc.scalar.activation(out=probs[:, off:off+w], in_=sps[:, 0:w],
                                                     func=AF.Exp, scale=SCALE, accum_out=acc)
                                accs.append(acc)
                                off += w
                            if len(accs) == 2:
                                nc.vector.tensor_add(out=accs[0], in0=accs[0], in1=accs[1])
                            rec = sm.tile([128, 1], F32, tag="rec", name="rec")
                            nc.vector.reciprocal(out=rec, in_=accs[0])
                            nc.vector.tensor_scalar_mul(out=probs[:, 0:kn], in0=probs[:, 0:kn], scalar1=rec)
                            attT = atp.tile([128, 1024], F16, tag=f"attT{si}", name=f"attT{si}", bufs=1)
                            pstr = ppt.tile([128, 1024], F16, tag="pstr", name="pstr")
                            for j in range(nblk):
                                nc.tensor.transpose(pstr[:, j*128:(j+1)*128],
                                                    probs[:, j*128:(j+1)*128], idn)
                            if si == 0:
                                nc.vector.tensor_copy(out=attT[:, 0:nblk*128],
                                                      in_=pstr[:, 0:nblk*128])
                            else:
                                nc.scalar.activation(out=attT[:, 0:nblk*128],
                                                     in_=pstr[:, 0:nblk*128], func=AF.Copy)
                            attTs.append(attT)
                        for si, g in enumerate(slots):
                            for bi in range(nblk):
                                b = kblocks[bi]
                                nc.tensor.matmul(ops[g // 8][(g % 2)*64:(g % 2)*64 + 64,
                                                             ((g // 2) % 4)*128:((g // 2) % 4 + 1)*128],
                                                 lhsT=v_full[:, b, kvh, :],
                                                 rhs=attTs[si][:, bi*128:(bi+1)*128],
                                                 start=(bi == 0), stop=(bi == nblk - 1),
                                                 tile_position=(0, (g % 2)*64))
                for t_ in range(2):
                    dstv = oT[:, t_*4:(t_+1)*4, qb*128:(qb+1)*128]
                    srcv = ops[t_].rearrange("p (f c) -> p f c", f=4)
                    if t_ == 0:
                        nc.vector.tensor_copy(out=dstv, in_=srcv)
                    else:
                        nc.scalar.activation(out=dstv, in_=srcv, func=AF.Copy)
            # ---- O projection ----
            wot = wp.tile([128, 8, 1024], F16, tag="wbig", name="wot", bufs=2)
            dma(out=wot, in_=P["wo"][l].rearrange("(ft p) c -> p ft c", p=128))
            for d in range(8):
                pso = ppm.tile([128, 256], F32, tag="pm", name="pso")
                for ft in range(8):
                    nc.tensor.matmul(pso, lhsT=wot[:, ft, d*128:(d+1)*128], rhs=oT[:, ft, :],
                                     start=(ft == 0), stop=(ft == 7))
                nc.vector.tensor_add(out=xT[d], in0=xT[d], in1=pso)
            # ---- FFN ----
            h2 = rmsnorm(P["n2h"][l])
            yT = []
            ci = 0
            for (c0, cw) in W13CH:
                w1t = wp.tile([128, 8, 512], F16, tag="w1c", name="w1c", bufs=2)
                w3t = wp.tile([128, 8, 512], F16, tag="w3c", name="w3c", bufs=2)
                dma(out=w1t[:, :, 0:cw], in_=P["w1"][l, :, c0:c0+cw].rearrange("(dc p) h -> p dc h", p=128))
                dma(out=w3t[:, :, 0:cw], in_=P["w3"][l, :, c0:c0+cw].rearrange("(dc p) h -> p dc h", p=128))
                for hc0 in range(0, cw, 128):
                    hw = min(128, cw - hc0)
                    psu = ppm.tile([128, 256], F32, tag="pm", name="psu")
                    psg = ppm.tile([128, 256], F32, tag="pm", name="psg")
                    for d in range(8):
                        nc.tensor.matmul(psu[0:hw, :], lhsT=w1t[:, d, hc0:hc0+hw], rhs=h2[d],
                                         start=(d == 0), stop=(d == 7))
                    for d in range(8):
                        nc.tensor.matmul(psg[0:hw, :], lhsT=w3t[:, d, hc0:hc0+hw], rhs=h2[d],
                                         start=(d == 0), stop=(d == 7))
                    su = act.tile([128, 256], F32, tag="su", name="su", bufs=1)
                    if SIM_SILU:
                        nc.scalar.activation(out=su[0:hw, :], in_=psu[0:hw, :], func=AF.Sigmoid)
                        nc.vector.tensor_mul(out=su[0:hw, :], in0=su[0:hw, :], in1=psu[0:hw, :])
                    else:
                        nc.scalar.activation(out=su[0:hw, :], in_=psu[0:hw, :], func=AF.Silu)
                    y = act.tile([128, 256], F16, tag=f"yT{ci}", name=f"yT{ci}", bufs=1)
                    nc.vector.tensor_mul(out=y[0:hw, :], in0=su[0:hw, :], in1=psg[0:hw, :])
                    yT.append(y)
                    ci += 1
            w2a = wp.tile([128, 11, 1024], F16, tag="w2a", name="w2a")
            w2b = wp.tile([128, 10, 1024], F16, tag="w2b", name="w2b")
            w2x = wp.tile([64, 1024], F16, tag="w2x", name="w2x")
            dma(out=w2a, in_=P["w2"][l, 0:1408, :].rearrange("(hc p) c -> p hc c", p=128))
            dma(out=w2b, in_=P["w2"][l, 1408:2688, :].rearrange("(hc p) c -> p hc c", p=128))
            dma(out=w2x, in_=P["w2"][l, 2688:2752, :])
            for d in range(8):
                ps2 = ppm.tile([128, 256], F32, tag="pm", name="ps2")
                for ci2, (h0, hwid) in enumerate(HCHUNKS):
                    if hwid == 64:
                        lh = w2x[:, d*128:(d+1)*128]
                    elif h0 < 1408:
                        lh = w2a[:, h0 // 128, d*128:(d+1)*128]
                    else:
                        lh = w2b[:, (h0 - 1408) // 128, d*128:(d+1)*128]
                    nc.tensor.matmul(ps2, lhsT=lh, rhs=yT[ci2][0:hwid, :],
                                     start=(ci2 == 0), stop=(ci2 == len(HCHUNKS) - 1))
                nc.vector.tensor_add(out=xT[d], in0=xT[d], in1=ps2)
        # ---- final norm + tied lm head ----
        xf = rmsnorm(P["nfh"][:, :])
        for d in range(8):
            dma(out=xf_in[d*128:(d+1)*128, :], in_=xf[d])
        if not SKIP_COLL:
            nc.gpsimd.collective_compute(
                "AllGather", mybir.AluOpType.bypass, replica_groups=RG_ALL,
                ins=[xf_in.ap()], outs=[xf_out.ap()])
        xfv = xf_out.rearrange("(r dc p) t -> p dc r t", p=128, dc=8)
        VCH = [(i * 256, 256) for i in range(15)] + [(3840, 160)]
        for th in range(2):
            xft = wp.tile([128, 8, 1024], F16, tag="wbig", name="xft", bufs=2)
            for d in range(8):
                dma(out=xft[:, d, :].rearrange("p (r t) -> p r t", r=4),
                    in_=xfv[:, d, th*4:(th+1)*4, :])
            for vci, (v0, vw) in enumerate(VCH):
                embt = wp.tile([128, 8, 256], F16, tag=("wkt", "wvt")[vci % 2], name="embt")
                dma(out=embt[:, :, 0:vw],
                    in_=P["embT"][:, v0:v0+vw].rearrange("(dc p) v -> p dc v", p=128))
                for vb0 in range(0, vw, 128) if not SKIP_LM else []:
                    vbw = min(128, vw - vb0)
                    psl = [ppb.tile([128, 512], F32, tag="pb", name="psl") for _ in range(2)]
                    for d in range(8):
                        for tb in range(2):
                            nc.tensor.matmul(psl[tb][0:vbw, :], lhsT=embt[:, d, vb0:vb0+vbw],
                                             rhs=xft[:, d, tb*512:(tb+1)*512],
                                             start=(d == 0), stop=(d == 7))
                    lg = act.tile([128, 1024], F16, tag=f"qtok{(vb0 // 128) % 2}", name="lg")
                    nc.vector.tensor_copy(out=lg[0:vbw, 0:512], in_=psl[0][0:vbw, :])
                    nc.scalar.activation(out=lg[0:vbw, 512:1024], in_=psl[1][0:vbw, :],
                                         func=AF.Copy)
                    dma(out=logits[v0+vb0:v0+vb0+vbw, th*1024:(th+1)*1024],
                        in_=lg[0:vbw, :])
    nc.compile()
    return nc

_NC_CACHE = {}
def _get_nc():
    if "nc" not in _NC_CACHE:
        _NC_CACHE["nc"] = _build_nc()
    return _NC_CACHE["nc"]

def kernel(**inputs):
    from concourse.bass_utils import run_bass_kernel_spmd
    nc = _get_nc()
    in_maps = _host_prep(**inputs)
    res = run_bass_kernel_spmd(nc, in_maps, list(range(NCORES)))
    return _assemble(res.results)


# revision 13
# speedup vs baseline: 1.6306x; 1.6306x over previous
import numpy as np, sys, os, math, functools
sys.path.insert(0, "/opt/trn_rl_repo")

V, D, L = 32000, 1024, 8
HQ, HKV, HD = 16, 4, 64
H = 2752
B, S = 2, 1024
WINDOW, GEVERY = 256, 4
EPS, BASE = 1e-6, 10000.0
NCORES = 8
T = 256
VSH = V // NCORES
NEG = -30000.0
SCALE = 1.0 / 8.0
SIM_SILU = False  # CoreSim lacks Silu; emulate with Sigmoid+mul when set
SKIP_COLL = False  # timing experiment: drop collectives
SKIP_LM = False    # timing experiment: drop lm-head compute
SKIP_ATTN = False  # timing experiment: drop attention score/AV work
SMALL_COLL = False # timing experiment: shrink collectives to latency-only

# slot g holds query head HPERM[g]; chosen so (g%2) == (HPERM[g]//4)%2,
# i.e. each head's partition parity matches its kv head's parity in kT.
HPERM = [0, 4, 1, 5, 2, 6, 3, 7, 8, 12, 9, 13, 10, 14, 11, 15]
KVH_OF_SLOT = [HPERM[g] // 4 for g in range(16)]
KVGROUPS = [[g for g in range(16) if KVH_OF_SLOT[g] == kvh] for kvh in range(4)]
# column block j of kT_full covers position block POSBLK[j]
POSBLK = [0, 1, 2, 3, 7, 6, 5, 4]
# w1/w3 DMA chunks and the derived 128-row hchunks
W13CH = [(0, 512), (512, 512), (1024, 512), (1536, 512), (2048, 512), (2560, 192)]
HCHUNKS = [(i * 128, 128) for i in range(21)] + [(2688, 64)]

def _core_blocks(c):
    cp = c % 4
    return cp, 7 - cp

def _inv_freq():
    return 1.0 / (BASE ** (np.arange(0, HD, 2, dtype=np.float64) / HD))

def _host_masks(c):
    bA, bB = _core_blocks(c)
    p = np.arange(128)
    qA = bA * 128 + p
    qB = bB * 128 + p
    posblk = np.asarray(POSBLK)
    def mk(qpos, c0, w, local):
        cols = c0 + np.arange(w)
        kpos = posblk[cols // 128] * 128 + cols % 128
        valid = kpos[None, :] <= qpos[:, None]
        if local:
            valid &= (qpos[:, None] - kpos[None, :]) < WINDOW
        return np.where(valid, 0.0, NEG).astype(np.float16)
    return (mk(qA, 0, 512, False), mk(qB, 512, 512, False),
            mk(qA, 0, 512, True), mk(qB, 256, 768, True))

def _host_prep(idx, emb, Wq, Wk, Wv, Wo, w1, w3, w2, n1, n2, nf):
    idx = np.asarray(idx); emb = np.asarray(emb, dtype=np.float32)
    invf = _inv_freq()
    # permute q heads (wq cols) and wo rows by HPERM
    Wq = np.asarray(Wq, dtype=np.float16).reshape(L, D, HQ, HD)
    Wq = np.ascontiguousarray(Wq[:, :, HPERM, :].reshape(L, D, HQ * HD))
    Wo = np.asarray(Wo, dtype=np.float16).reshape(L, HQ, HD, D)
    Wo = np.ascontiguousarray(Wo[:, HPERM, :, :].reshape(L, HQ * HD, D))
    wk16 = np.asarray(Wk, dtype=np.float16)
    wv16 = np.asarray(Wv, dtype=np.float16)
    w116 = np.asarray(w1, dtype=np.float16)
    w316 = np.asarray(w3, dtype=np.float16)
    w216 = np.asarray(w2, dtype=np.float16)
    n1h = np.ascontiguousarray(np.asarray(n1).reshape(L, 8, 128).transpose(0, 2, 1))
    n2h = np.ascontiguousarray(np.asarray(n2).reshape(L, 8, 128).transpose(0, 2, 1))
    nfh = np.ascontiguousarray(np.asarray(nf).reshape(8, 128).T)
    in_maps = []
    for c in range(NCORES):
        s = c // 4
        bA, bB = _core_blocks(c)
        tok = np.concatenate([idx[s, bA*128:(bA+1)*128], idx[s, bB*128:(bB+1)*128]])
        x0T = np.ascontiguousarray(emb[tok].T)
        pos = np.concatenate([bA*128 + np.arange(128), bB*128 + np.arange(128)])
        ang = pos[:, None].astype(np.float64) * invf[None, :]
        cosq = np.cos(ang).astype(np.float32).reshape(2, 128, 32)
        sinq = np.sin(ang).astype(np.float32).reshape(2, 128, 32)
        mAg, mBg, mAl, mBl = _host_masks(c)
        embT = np.ascontiguousarray(emb[c*VSH:(c+1)*VSH].T.astype(np.float16))
        in_maps.append({
            "x0T": x0T,
            "wq": Wq, "wk": wk16, "wv": wv16, "wo": Wo,
            "w1": w116, "w3": w316, "w2": w216,
            "n1h": n1h, "n2h": n2h, "nfh": nfh,
            "cosq": cosq, "sinq": sinq,
            "mAg": mAg, "mBg": mBg, "mAl": mAl, "mBl": mBl,
            "embT": embT,
        })
    return in_maps

def _unperm_rows():
    perm = np.zeros(2048, dtype=np.int64)
    for r in range(2048):
        rr, rem = divmod(r, 256)
        slot, p = divmod(rem, 128)
        samp = rr // 4
        bA, bB = _core_blocks(rr)
        blk = bA if slot == 0 else bB
        perm[r] = samp * S + blk * 128 + p
    inv = np.zeros(2048, dtype=np.int64)
    inv[perm] = np.arange(2048)
    return inv

def _assemble(outs):
    fullT = np.empty((V, 2048), dtype=np.float32)
    for c in range(NCORES):
        fullT[c*VSH:(c+1)*VSH, :] = outs[c]["logits"]
    full = np.ascontiguousarray(fullT.T)
    inv = _unperm_rows()
    return full[inv].reshape(B, S, V)

def _build_nc():
    import concourse.bass as bass
    import concourse.bacc as bacc
    import concourse.mybir as mybir
    from concourse.tile import TileContext
    from concourse.masks import make_identity
    F32, F16 = mybir.dt.float32, mybir.dt.float16
    AF = mybir.ActivationFunctionType
    ALU = mybir.AluOpType

    nc = bacc.Bacc("TRN2", target_bir_lowering=False, debug=False, num_devices=NCORES)
    P = {}
    def inp(name, shape, dt=F32):
        P[name] = nc.declare_dram_parameter(name, list(shape), dt, isOutput=False)
    inp("x0T", (D, T))
    inp("wq", (L, D, D), F16); inp("wk", (L, D, 256), F16); inp("wv", (L, D, 256), F16)
    inp("wo", (L, D, D), F16)
    inp("w1", (L, D, H), F16); inp("w3", (L, D, H), F16); inp("w2", (L, H, D), F16)
    inp("n1h", (L, 128, 8)); inp("n2h", (L, 128, 8)); inp("nfh", (128, 8))
    inp("cosq", (2, 128, 32)); inp("sinq", (2, 128, 32))
    inp("mAg", (128, 512), F16); inp("mBg", (128, 512), F16)
    inp("mAl", (128, 512), F16); inp("mBl", (128, 768), F16)
    inp("embT", (D, VSH), F16)
    logits = nc.declare_dram_parameter("logits", [VSH, 2048], F16, isOutput=True)

    kv_in  = [nc.dram_tensor(f"kv_in{l}",  [4, 128, 256], F16) for l in range(L)]
    kv_out = [nc.dram_tensor(f"kv_out{l}", [16, 128, 256], F16) for l in range(L)]
    if SMALL_COLL:
        kv_in_s  = [nc.dram_tensor(f"kv_in_s{l}",  [4, 1, 256], F16) for l in range(L)]
        kv_out_s = [nc.dram_tensor(f"kv_out_s{l}", [16, 1, 256], F16) for l in range(L)]
        xf_in_s  = nc.dram_tensor("xf_in_s", [8, T], F16)
        xf_out_s = nc.dram_tensor("xf_out_s", [64, T], F16, addr_space="Shared")
    xf_in  = nc.dram_tensor("xf_in", [D, T], F16)
    xf_out = nc.dram_tensor("xf_out", [NCORES * D, T], F16, addr_space="Shared")
    RG_KV = [[0, 1, 2, 3], [4, 5, 6, 7]]
    RG_ALL = [list(range(NCORES))]

    with TileContext(nc) as tc:
      with tc.tile_pool(name="pers", bufs=1) as pers, \
           tc.tile_pool(name="wpool", bufs=1) as wp, \
           tc.tile_pool(name="act", bufs=1) as act, \
           tc.tile_pool(name="attn", bufs=1) as atp, \
           tc.tile_pool(name="small", bufs=4) as sm, \
           tc.tile_pool(name="ppb", bufs=2, space="PSUM") as ppb, \
           tc.tile_pool(name="ppo", bufs=2, space="PSUM") as ppo, \
           tc.tile_pool(name="ppm", bufs=3, space="PSUM") as ppm, \
           tc.tile_pool(name="ppt", bufs=1, space="PSUM") as ppt:

        dma = nc.sync.dma_start
        xT = [pers.tile([128, T], F32, tag=f"xT{d}", name=f"xT{d}") for d in range(8)]
        for d in range(8):
            dma(out=xT[d], in_=P["x0T"][d*128:(d+1)*128, :])
        cosA = pers.tile([128, 32], F32, tag="cosA", name="cosA"); dma(out=cosA, in_=P["cosq"][0])
        cosB = pers.tile([128, 32], F32, tag="cosB", name="cosB"); dma(out=cosB, in_=P["cosq"][1])
        sinA = pers.tile([128, 32], F32, tag="sinA", name="sinA"); dma(out=sinA, in_=P["sinq"][0])
        sinB = pers.tile([128, 32], F32, tag="sinB", name="sinB"); dma(out=sinB, in_=P["sinq"][1])
        mAg = pers.tile([128, 512], F16, tag="mAg", name="mAg"); dma(out=mAg, in_=P["mAg"][:, :])
        mBg = pers.tile([128, 512], F16, tag="mBg", name="mBg"); dma(out=mBg, in_=P["mBg"][:, :])
        mAl = pers.tile([128, 512], F16, tag="mAl", name="mAl"); dma(out=mAl, in_=P["mAl"][:, :])
        mBl = pers.tile([128, 768], F16, tag="mBl", name="mBl"); dma(out=mBl, in_=P["mBl"][:, :])
        idn = pers.tile([128, 128], F16, tag="idn", name="idn")
        make_identity(nc, idn)
        ones16 = pers.tile([128, 1], F16, tag="ones16", name="ones16")
        nc.vector.memset(ones16, 1.0)
        epst = pers.tile([1, 1], F32, tag="epst", name="epst")
        nc.vector.memset(epst, EPS)

        def rmsnorm(nw_dram):
            nw = sm.tile([128, 8], F32, tag="nw", name="nw")
            dma(out=nw, in_=nw_dram)
            ssp = ppm.tile([128, T], F32, tag="pm", name="ssp")
            for d in range(8):
                x2 = act.tile([128, T], F16, tag="x2", name="x2", bufs=2)
                nc.vector.tensor_mul(out=x2, in0=xT[d], in1=xT[d])
                nc.tensor.matmul(ssp[0:1, :], lhsT=ones16, rhs=x2, start=(d == 0), stop=(d == 7))
            lnr = sm.tile([1, T], F32, tag="lnr", name="lnr")
            nc.scalar.activation(out=lnr, in_=ssp[0:1, :], func=AF.Ln, scale=1.0/D,
                                 bias=epst[0:1, 0:1])
            rr = sm.tile([1, T], F32, tag="rr", name="rr")
            nc.scalar.activation(out=rr, in_=lnr, func=AF.Exp, scale=-0.5)
            rb = act.tile([128, T], F32, tag="rb", name="rb")
            nc.gpsimd.partition_broadcast(rb[:], rr[:])
            out = []
            for d in range(8):
                h = act.tile([128, T], F16, tag=f"hT{d}", name=f"hT{d}", bufs=1)
                nc.vector.scalar_tensor_tensor(out=h, in0=xT[d], scalar=nw[:, d:d+1],
                                               in1=rb, op0=ALU.mult, op1=ALU.mult)
                out.append(h)
            return out

        def rope_tok(ps, cost, sint, outt, nheads):
            ev = ps.rearrange("p (h f two) -> p h f two", two=2, f=32)
            ov = outt.rearrange("p (h f two) -> p h f two", two=2, f=32)
            cb = cost[:].rearrange("p (o f) -> p o f", o=1).to_broadcast((128, nheads, 32))
            sb = sint[:].rearrange("p (o f) -> p o f", o=1).to_broadcast((128, nheads, 32))
            t1 = sm.tile([128, nheads, 32], F32, tag="ropet1", name="ropet1", bufs=2)
            t2 = sm.tile([128, nheads, 32], F32, tag="ropet2", name="ropet2", bufs=2)
            nc.vector.tensor_mul(out=t1, in0=ev[:, :, :, 0], in1=cb)
            nc.vector.tensor_mul(out=t2, in0=ev[:, :, :, 1], in1=sb)
            nc.vector.tensor_sub(out=ov[:, :, :, 0], in0=t1, in1=t2)
            nc.vector.tensor_mul(out=t1, in0=ev[:, :, :, 0], in1=sb)
            nc.vector.tensor_mul(out=t2, in0=ev[:, :, :, 1], in1=cb)
            nc.vector.tensor_add(out=ov[:, :, :, 1], in0=t1, in1=t2)

        for l in range(L):
            is_global = ((l + 1) % GEVERY) == 0
            h1 = rmsnorm(P["n1h"][l])
            # ---- QKV projections (shared stationary h1 slices) ----
            wqt = wp.tile([128, 8, 1024], F16, tag="wbig", name="wqt", bufs=2)
            dma(out=wqt, in_=P["wq"][l].rearrange("(dc p) f -> p dc f", p=128))
            wkt = wp.tile([128, 8, 256], F16, tag="wkt", name="wkt")
            wvt = wp.tile([128, 8, 256], F16, tag="wvt", name="wvt")
            dma(out=wkt, in_=P["wk"][l].rearrange("(dc p) f -> p dc f", p=128))
            dma(out=wvt, in_=P["wv"][l].rearrange("(dc p) f -> p dc f", p=128))
            ktok = act.tile([128, 512], F16, tag="ktok", name="ktok")
            vtok = act.tile([128, 512], F16, tag="vtok", name="vtok")
            qtoks = [act.tile([128, 1024], F16, tag=f"qtok{t}", name=f"qtok{t}", bufs=1) for t in range(2)]
            for t2_ in range(2):
                psk = ppm.tile([128, 256], F32, tag="pm", name="psk")
                psv = ppm.tile([128, 256], F32, tag="pm", name="psv")
                psq0 = ppb.tile([128, 512], F32, tag="pb", name="psq0")
                psq1 = ppb.tile([128, 512], F32, tag="pb", name="psq1")
                for d in range(8):
                    lh = h1[d][:, t2_*128:(t2_+1)*128]
                    st, sp = (d == 0), (d == 7)
                    nc.tensor.matmul(psk, lhsT=lh, rhs=wkt[:, d, :], start=st, stop=sp)
                    nc.tensor.matmul(psv, lhsT=lh, rhs=wvt[:, d, :], start=st, stop=sp)
                    nc.tensor.matmul(psq0, lhsT=lh, rhs=wqt[:, d, 0:512], start=st, stop=sp)
                    nc.tensor.matmul(psq1, lhsT=lh, rhs=wqt[:, d, 512:1024], start=st, stop=sp)
                rope_tok(psk, (cosA, cosB)[t2_], (sinA, sinB)[t2_], ktok[:, t2_*256:(t2_+1)*256], 4)
                nc.scalar.activation(out=vtok[:, t2_*256:(t2_+1)*256], in_=psv, func=AF.Copy)
                rope_tok(psq0, (cosA, cosB)[t2_], (sinA, sinB)[t2_], qtoks[t2_][:, 0:512], 8)
                rope_tok(psq1, (cosA, cosB)[t2_], (sinA, sinB)[t2_], qtoks[t2_][:, 512:1024], 8)
            # ---- transpose k (4 blocks) into kv_in layout ----
            kTsb = act.tile([128, 512], F16, tag="kTsb", name="kTsb")
            pstk = ppt.tile([128, 1024], F16, tag="pstr", name="pstk")
            for i in range(2):
                for t2_ in range(2):
                    nc.tensor.transpose(pstk[:, i*256 + t2_*128:i*256 + (t2_+1)*128],
                                        ktok[:, t2_*256 + i*128:t2_*256 + (i+1)*128], idn)
            nc.scalar.activation(out=kTsb, in_=pstk[:, 0:512], func=AF.Copy)
            for i in range(2):
                dma(out=kv_in[l][i], in_=kTsb[:, i*256:(i+1)*256])
                dma(out=kv_in[l][2 + i], in_=vtok[:, i*256:(i+1)*256])
            if SMALL_COLL:
                dma(out=kv_in_s[l][0, :, :], in_=kTsb[0:1, 0:256])
                nc.gpsimd.collective_compute(
                    "AllGather", mybir.AluOpType.bypass, replica_groups=RG_KV,
                    ins=[kv_in_s[l].ap()], outs=[kv_out_s[l].ap()])

            elif not SKIP_COLL:
                nc.gpsimd.collective_compute(
                    "AllGather", mybir.AluOpType.bypass, replica_groups=RG_KV,
                    ins=[kv_in[l].ap()], outs=[kv_out[l].ap()])
            # ---- transpose q (16 blocks) -> qT [128, 8, 256] ----
            qT = atp.tile([128, 8, 256], F16, tag="qT", name="qT")
            for t2_ in range(2):
                pst = ppt.tile([128, 1024], F16, tag="pstr", name="pst")
                for f in range(8):
                    nc.tensor.transpose(pst[:, f*128:(f+1)*128],
                                        qtoks[t2_][:, f*128:(f+1)*128], idn)
                dstv = qT[:, :, t2_*128:(t2_+1)*128]
                srcv = pst.rearrange("p (f c) -> p f c", f=8)
                if t2_ == 0:
                    nc.vector.tensor_copy(out=dstv, in_=srcv)
                else:
                    nc.scalar.activation(out=dstv, in_=srcv, func=AF.Copy)
            # ---- gather K/V from collective ----
            kT_full = [atp.tile([128, 1024], F16, tag=f"kTf{i}", name=f"kTf{i}", bufs=1) for i in range(2)]
            kvo = kv_out[l].rearrange("(r x) p c -> x r p c", x=4)
            for i in range(2):
                dma(out=kT_full[i].rearrange("p (sl r c) -> p sl r c", sl=2, r=4),
                    in_=kvo[i].rearrange("r p (sl c) -> p sl r c", sl=2))
            if SMALL_COLL:
                dma(out=kT_full[0][0:1, 0:256], in_=kv_out_s[l][0])
            v_full = atp.tile([128, 8, 4, 64], F16, tag="vfull", name="vfull")
            for sl in range(2):
                dma(out=v_full[:, sl*4:(sl+1)*4, :, :],
                    in_=kvo[2 + sl].rearrange("r p (h f) -> p r h f", f=64))
            # ---- attention ----
            oT = atp.tile([128, 8, 256], F16, tag="oT", name="oT")
            for qb in range(2):
                if qb == 0:
                    chunks = [(0, 512, mAg if is_global else mAl)]
                    kblocks = [0, 1, 2, 3]
                elif is_global:
                    chunks = [(0, 512, None), (512, 512, mBg)]
                    kblocks = list(range(8))
                else:
                    chunks = [(256, 512, mBl[:, 0:512]), (768, 256, mBl[:, 512:768])]
                    kblocks = [2, 3, 4, 5, 6, 7]
                kn = sum(w for _, w, _ in chunks)
                nblk = len(kblocks)
                ops = [ppo.tile([128, 512], F32, tag=f"ops{t}", name=f"ops{t}", bufs=1) for t in range(2)]
                for kvh in range(4 if not SKIP_ATTN else 0):
                    ktile = kT_full[kvh // 2]
                    for half in range(2):
                        slots = KVGROUPS[kvh][half*2:(half+1)*2]
                        attTs = []
                        for si, g in enumerate(slots):
                            base = (g % 2) * 64
                            lhq = qT[base:base+64, g//2, qb*128:(qb+1)*128]
                            probs = atp.tile([128, 1024], F16, tag=f"probs{si}", name=f"probs{si}", bufs=1)
                            spss = []
                            for (c0, w, msk) in chunks:
                                sps = ppb.tile([128, 512], F32, tag="pb", name="sps")
                                nc.tensor.matmul(sps[:, 0:w], lhsT=lhq,
                                                 rhs=ktile[base:base+64, c0:c0+w],
                                                 start=True, stop=True)
                                spss.append(sps)
                            for sps, (c0, w, msk) in zip(spss, chunks):
                                if msk is not None:
                                    nc.vector.tensor_add(out=sps[:, 0:w], in0=sps[:, 0:w], in1=msk)
                            off = 0
                            accs = []
                            for sps, (c0, w, msk) in zip(spss, chunks):
                                acc = sm.tile([128, 1], F32, tag="acc", name="acc")
                                nc.scalar.activation(out=probs[:, off:off+w], in_=sps[:, 0:w],
                                                     func=AF.Exp, scale=SCALE, accum_out=acc)
                                accs.append(acc)
                                off += w
                            if len(accs) == 2:
                                nc.vector.tensor_add(out=accs[0], in0=accs[0], in1=accs[1])
                            rec = sm.tile([128, 1], F32, tag="rec", name="rec")
                            nc.vector.reciprocal(out=rec, in_=accs[0])
                            nc.vector.tensor_scalar_mul(out=probs[:, 0:kn], in0=probs[:, 0:kn], scalar1=rec)
                            attT = atp.tile([128, 1024], F16, tag=f"attT{si}", name=f"attT{si}", bufs=1)
                            pstr = ppt.tile([128, 1024], F16, tag="pstr", name="pstr")
                            for j in range(nblk):
                                nc.tensor.transpose(pstr[:, j*128:(j+1)*128],
                                                    probs[:, j*128:(j+1)*128], idn)
                            if si == 0:
                                nc.vector.tensor_copy(out=attT[:, 0:nblk*128],
                                                      in_=pstr[:, 0:nblk*128])
                            else:
                                nc.scalar.activation(out=attT[:, 0:nblk*128],
                                                     in_=pstr[:, 0:nblk*128], func=AF.Copy)
                            attTs.append(attT)
                        for si, g in enumerate(slots):
                            for bi in range(nblk):
                                b = kblocks[bi]
                                nc.tensor.matmul(ops[g // 8][(g % 2)*64:(g % 2)*64 + 64,
                                                             ((g // 2) % 4)*128:((g // 2) % 4 + 1)*128],
                                                 lhsT=v_full[:, b, kvh, :],
                                                 rhs=attTs[si][:, bi*128:(bi+1)*128],
                                                 start=(bi == 0), stop=(bi == nblk - 1),
                                                 tile_position=(0, (g % 2)*64))
                for t_ in range(2):
                    dstv = oT[:, t_*4:(t_+1)*4, qb*128:(qb+1)*128]
                    srcv = ops[t_].rearrange("p (f c) -> p f c", f=4)
                    if t_ == 0:
                        nc.vector.tensor_copy(out=dstv, in_=srcv)
                    else:
                        nc.scalar.activation(out=dstv, in_=srcv, func=AF.Copy)
            # ---- O projection ----
            wot = wp.tile([128, 8, 1024], F16, tag="wbig", name="wot", bufs=2)
            dma(out=wot, in_=P["wo"][l].rearrange("(ft p) c -> p ft c", p=128))
            for d in range(8):
                pso = ppm.tile([128, 256], F32, tag="pm", name="pso")
                for ft in range(8):
                    nc.tensor.matmul(pso, lhsT=wot[:, ft, d*128:(d+1)*128], rhs=oT[:, ft, :],
                                     start=(ft == 0), stop=(ft == 7))
                nc.vector.tensor_add(out=xT[d], in0=xT[d], in1=pso)
            # ---- FFN ----
            h2 = rmsnorm(P["n2h"][l])
            yT = []
            ci = 0
            for (c0, cw) in W13CH:
                w1t = wp.tile([128, 8, 512], F16, tag="w1c", name="w1c", bufs=2)
                w3t = wp.tile([128, 8, 512], F16, tag="w3c", name="w3c", bufs=2)
                dma(out=w1t[:, :, 0:cw], in_=P["w1"][l, :, c0:c0+cw].rearrange("(dc p) h -> p dc h", p=128))
                dma(out=w3t[:, :, 0:cw], in_=P["w3"][l, :, c0:c0+cw].rearrange("(dc p) h -> p dc h", p=128))
                for hc0 in range(0, cw, 128):
                    hw = min(128, cw - hc0)
                    psu = ppm.tile([128, 256], F32, tag="pm", name="psu")
                    psg = ppm.tile([128, 256], F32, tag="pm", name="psg")
                    for d in range(8):
                        nc.tensor.matmul(psu[0:hw, :], lhsT=w1t[:, d, hc0:hc0+hw], rhs=h2[d],
                                         start=(d == 0), stop=(d == 7))
                    for d in range(8):
                        nc.tensor.matmul(psg[0:hw, :], lhsT=w3t[:, d, hc0:hc0+hw], rhs=h2[d],
                                         start=(d == 0), stop=(d == 7))
                    su = act.tile([128, 256], F32, tag="su", name="su", bufs=1)
                    if SIM_SILU:
                        nc.scalar.activation(out=su[0:hw, :], in_=psu[0:hw, :], func=AF.Sigmoid)
                        nc.vector.tensor_mul(out=su[0:hw, :], in0=su[0:hw, :], in1=psu[0:hw, :])
                    else:
                        nc.scalar.activation(out=su[0:hw, :], in_=psu[0:hw, :], func=AF.Silu)
                    y = act.tile([128, 256], F16, tag=f"yT{ci}", name=f"yT{ci}", bufs=1)
                    nc.vector.tensor_mul(out=y[0:hw, :], in0=su[0:hw, :], in1=psg[0:hw, :])
                    yT.append(y)
                    ci += 1
            w2a = wp.tile([128, 11, 1024], F16, tag="w2a", name="w2a")
            w2b = wp.tile([128, 10, 1024], F16, tag="w2b", name="w2b")
            w2x = wp.tile([64, 1024], F16, tag="w2x", name="w2x")
            dma(out=w2a, in_=P["w2"][l, 0:1408, :].rearrange("(hc p) c -> p hc c", p=128))
            dma(out=w2b, in_=P["w2"][l, 1408:2688, :].rearrange("(hc p) c -> p hc c", p=128))
            dma(out=w2x, in_=P["w2"][l, 2688:2752, :])
            for d in range(8):
                ps2 = ppm.tile([128, 256], F32, tag="pm", name="ps2")
                for ci2, (h0, hwid) in enumerate(HCHUNKS):
                    if hwid == 64:
                        lh = w2x[:, d*128:(d+1)*128]
                    elif h0 < 1408:
                        lh = w2a[:, h0 // 128, d*128:(d+1)*128]
                    else:
                        lh = w2b[:, (h0 - 1408) // 128, d*128:(d+1)*128]
                    nc.tensor.matmul(ps2, lhsT=lh, rhs=yT[ci2][0:hwid, :],
                                     start=(ci2 == 0), stop=(ci2 == len(HCHUNKS) - 1))
                nc.vector.tensor_add(out=xT[d], in0=xT[d], in1=ps2)
        # ---- final norm + tied lm head ----
        xf = rmsnorm(P["nfh"][:, :])
        for d in range(8):
            dma(out=xf_in[d*128:(d+1)*128, :], in_=xf[d])
        if SMALL_COLL:
            dma(out=xf_in_s[:, :], in_=xf[0][0:8, :])
            nc.gpsimd.collective_compute(
                "AllGather", mybir.AluOpType.bypass, replica_groups=RG_ALL,
                ins=[xf_in_s.ap()], outs=[xf_out_s.ap()])
        elif not SKIP_COLL:
            nc.gpsimd.collective_compute(
                "AllGather", mybir.AluOpType.bypass, replica_groups=RG_ALL,
                ins=[xf_in.ap()], outs=[xf_out.ap()])
        xfv = xf_out.rearrange("(r dc p) t -> p dc r t", p=128, dc=8)
        VCH = [(i * 256, 256) for i in range(15)] + [(3840, 160)]
        for th in range(2):
            xft = wp.tile([128, 8, 1024], F16, tag="wbig", name="xft", bufs=2)
            for d in range(8):
                dma(out=xft[:, d, :].rearrange("p (r t) -> p r t", r=4),
                    in_=xfv[:, d, th*4:(th+1)*4, :])
            if SMALL_COLL:
                dma(out=xft[0:1, 0, 0:64], in_=xf_out_s[0, 0:64])
            for vci, (v0, vw) in enumerate(VCH):
                embt = wp.tile([128, 8, 256], F16, tag=("wkt", "wvt")[vci % 2], name="embt")
                dma(out=embt[:, :, 0:vw],
                    in_=P["embT"][:, v0:v0+vw].rearrange("(dc p) v -> p dc v", p=128))
                for vb0 in range(0, vw, 128) if not SKIP_LM else []:
                    vbw = min(128, vw - vb0)
                    psl = [ppb.tile([128, 512], F32, tag="pb", name="psl") for _ in range(2)]
                    for d in range(8):
                        for tb in range(2):
                            nc.tensor.matmul(psl[tb][0:vbw, :], lhsT=embt[:, d, vb0:vb0+vbw],
                                             rhs=xft[:, d, tb*512:(tb+1)*512],
                                             start=(d == 0), stop=(d == 7))
                    lg = act.tile([128, 1024], F16, tag=f"qtok{(vb0 // 128) % 2}", name="lg")
                    nc.vector.tensor_copy(out=lg[0:vbw, 0:512], in_=psl[0][0:vbw, :])
                    nc.scalar.activation(out=lg[0:vbw, 512:1024], in_=psl[1][0:vbw, :],
                                         func=AF.Copy)
                    dma(out=logits[v0+vb0:v0+vb0+vbw, th*1024:(th+1)*1024],
                        in_=lg[0:vbw, :])
    nc.compile()
    return nc

_NC_CACHE = {}
def _get_nc():
    if "nc" not in _NC_CACHE:
        _NC_CACHE["nc"] = _build_nc()
    return _NC_CACHE["nc"]

def kernel(**inputs):
    from concourse.bass_utils import run_bass_kernel_spmd
    nc = _get_nc()
    in_maps = _host_prep(**inputs)
    res = run_bass_kernel_spmd(nc, in_maps, list(range(NCORES)))
    return _assemble(res.results)


# revision 14
# speedup vs baseline: 4.3976x; 2.6970x over previous
import numpy as np, sys, os, math, functools
sys.path.insert(0, "/opt/trn_rl_repo")

V, D, L = 32000, 1024, 8
HQ, HKV, HD = 16, 4, 64
H = 2752
B, S = 2, 1024
WINDOW, GEVERY = 256, 4
EPS, BASE = 1e-6, 10000.0
NCORES = 8
T = 256
VSH = V // NCORES
NEG = -30000.0
SCALE = 1.0 / 8.0
SIM_SILU = False  # CoreSim lacks Silu; emulate with Sigmoid+mul when set
SKIP_COLL = False  # timing experiment: drop collectives
SKIP_LM = False    # timing experiment: drop lm-head compute
SKIP_ATTN = False  # timing experiment: drop attention score/AV work
SMALL_COLL = False # timing experiment: shrink collectives to latency-only

# slot g holds query head HPERM[g]; chosen so (g%2) == (HPERM[g]//4)%2,
# i.e. each head's partition parity matches its kv head's parity in kT.
HPERM = [0, 4, 1, 5, 2, 6, 3, 7, 8, 12, 9, 13, 10, 14, 11, 15]
KVH_OF_SLOT = [HPERM[g] // 4 for g in range(16)]
KVGROUPS = [[g for g in range(16) if KVH_OF_SLOT[g] == kvh] for kvh in range(4)]
# column block j of kT_full covers position block POSBLK[j]
POSBLK = [0, 1, 2, 3, 7, 6, 5, 4]
# w1/w3 DMA chunks and the derived 128-row hchunks
W13CH = [(0, 512), (512, 512), (1024, 512), (1536, 512), (2048, 512), (2560, 192)]
HCHUNKS = [(i * 128, 128) for i in range(21)] + [(2688, 64)]

def _core_blocks(c):
    cp = c % 4
    return cp, 7 - cp

def _inv_freq():
    return 1.0 / (BASE ** (np.arange(0, HD, 2, dtype=np.float64) / HD))

def _host_masks(c):
    bA, bB = _core_blocks(c)
    p = np.arange(128)
    qA = bA * 128 + p
    qB = bB * 128 + p
    posblk = np.asarray(POSBLK)
    def mk(qpos, c0, w, local):
        cols = c0 + np.arange(w)
        kpos = posblk[cols // 128] * 128 + cols % 128
        valid = kpos[None, :] <= qpos[:, None]
        if local:
            valid &= (qpos[:, None] - kpos[None, :]) < WINDOW
        return np.where(valid, 0.0, NEG).astype(np.float16)
    return (mk(qA, 0, 512, False), mk(qB, 512, 512, False),
            mk(qA, 0, 512, True), mk(qB, 256, 768, True))

def _host_prep(idx, emb, Wq, Wk, Wv, Wo, w1, w3, w2, n1, n2, nf):
    idx = np.asarray(idx); emb = np.asarray(emb, dtype=np.float32)
    invf = _inv_freq()
    # permute q heads (wq cols) and wo rows by HPERM
    Wq = np.asarray(Wq, dtype=np.float16).reshape(L, D, HQ, HD)
    Wq = np.ascontiguousarray(Wq[:, :, HPERM, :].reshape(L, D, HQ * HD))
    Wo = np.asarray(Wo, dtype=np.float16).reshape(L, HQ, HD, D)
    Wo = np.ascontiguousarray(Wo[:, HPERM, :, :].reshape(L, HQ * HD, D))
    wk16 = np.asarray(Wk, dtype=np.float16)
    wv16 = np.asarray(Wv, dtype=np.float16)
    w116 = np.asarray(w1, dtype=np.float16)
    w316 = np.asarray(w3, dtype=np.float16)
    w216 = np.asarray(w2, dtype=np.float16)
    n1h = np.ascontiguousarray(np.asarray(n1).reshape(L, 8, 128).transpose(0, 2, 1))
    n2h = np.ascontiguousarray(np.asarray(n2).reshape(L, 8, 128).transpose(0, 2, 1))
    nfh = np.ascontiguousarray(np.asarray(nf).reshape(8, 128).T)
    in_maps = []
    for c in range(NCORES):
        s = c // 4
        bA, bB = _core_blocks(c)
        tok = np.concatenate([idx[s, bA*128:(bA+1)*128], idx[s, bB*128:(bB+1)*128]])
        x0T = np.ascontiguousarray(emb[tok].T)
        pos = np.concatenate([bA*128 + np.arange(128), bB*128 + np.arange(128)])
        ang = pos[:, None].astype(np.float64) * invf[None, :]
        cosq = np.cos(ang).astype(np.float32).reshape(2, 128, 32)
        sinq = np.sin(ang).astype(np.float32).reshape(2, 128, 32)
        mAg, mBg, mAl, mBl = _host_masks(c)
        embT = np.ascontiguousarray(emb[c*VSH:(c+1)*VSH].T.astype(np.float16))
        in_maps.append({
            "x0T": x0T,
            "wq": Wq, "wk": wk16, "wv": wv16, "wo": Wo,
            "w1": w116, "w3": w316, "w2": w216,
            "n1h": n1h, "n2h": n2h, "nfh": nfh,
            "cosq": cosq, "sinq": sinq,
            "mAg": mAg, "mBg": mBg, "mAl": mAl, "mBl": mBl,
            "embT": embT,
        })
    return in_maps

def _unperm_rows():
    perm = np.zeros(2048, dtype=np.int64)
    for r in range(2048):
        rr, rem = divmod(r, 256)
        slot, p = divmod(rem, 128)
        samp = rr // 4
        bA, bB = _core_blocks(rr)
        blk = bA if slot == 0 else bB
        perm[r] = samp * S + blk * 128 + p
    inv = np.zeros(2048, dtype=np.int64)
    inv[perm] = np.arange(2048)
    return inv

def _assemble(outs):
    fullT = np.empty((V, 2048), dtype=np.float32)
    for c in range(NCORES):
        fullT[c*VSH:(c+1)*VSH, :] = outs[c]["logits"]
    full = np.ascontiguousarray(fullT.T)
    inv = _unperm_rows()
    return full[inv].reshape(B, S, V)

def _build_nc():
    import concourse.bass as bass
    import concourse.bacc as bacc
    import concourse.mybir as mybir
    from concourse.tile import TileContext
    from concourse.masks import make_identity
    F32, F16 = mybir.dt.float32, mybir.dt.float16
    AF = mybir.ActivationFunctionType
    ALU = mybir.AluOpType

    nc = bacc.Bacc("TRN2", target_bir_lowering=False, debug=False, num_devices=NCORES)
    P = {}
    def inp(name, shape, dt=F32):
        P[name] = nc.declare_dram_parameter(name, list(shape), dt, isOutput=False)
    inp("x0T", (D, T))
    inp("wq", (L, D, D), F16); inp("wk", (L, D, 256), F16); inp("wv", (L, D, 256), F16)
    inp("wo", (L, D, D), F16)
    inp("w1", (L, D, H), F16); inp("w3", (L, D, H), F16); inp("w2", (L, H, D), F16)
    inp("n1h", (L, 128, 8)); inp("n2h", (L, 128, 8)); inp("nfh", (128, 8))
    inp("cosq", (2, 128, 32)); inp("sinq", (2, 128, 32))
    inp("mAg", (128, 512), F16); inp("mBg", (128, 512), F16)
    inp("mAl", (128, 512), F16); inp("mBl", (128, 768), F16)
    inp("embT", (D, VSH), F16)
    logits = nc.declare_dram_parameter("logits", [VSH, 2048], F16, isOutput=True)

    kv_in  = [nc.dram_tensor(f"kv_in{l}",  [4, 128, 256], F16) for l in range(L)]
    kv_out = [nc.dram_tensor(f"kv_out{l}", [16, 128, 256], F16) for l in range(L)]
    if SMALL_COLL:
        kv_in_s  = [nc.dram_tensor(f"kv_in_s{l}",  [4, 1, 256], F16) for l in range(L)]
        kv_out_s = [nc.dram_tensor(f"kv_out_s{l}", [16, 1, 256], F16) for l in range(L)]
        xf_in_s  = nc.dram_tensor("xf_in_s", [8, T], F16)
        xf_out_s = nc.dram_tensor("xf_out_s", [64, T], F16, addr_space="Shared")
    xf_in  = nc.dram_tensor("xf_in", [D, T], F16)
    xf_out = nc.dram_tensor("xf_out", [NCORES * D, T], F16, addr_space="Shared")
    RG_KV = [[0, 1, 2, 3], [4, 5, 6, 7]]
    RG_ALL = [list(range(NCORES))]

    with TileContext(nc) as tc:
      with tc.tile_pool(name="pers", bufs=1) as pers, \
           tc.tile_pool(name="wpool", bufs=1) as wp, \
           tc.tile_pool(name="act", bufs=1) as act, \
           tc.tile_pool(name="attn", bufs=1) as atp, \
           tc.tile_pool(name="small", bufs=4) as sm, \
           tc.tile_pool(name="ppb", bufs=2, space="PSUM") as ppb, \
           tc.tile_pool(name="ppo", bufs=2, space="PSUM") as ppo, \
           tc.tile_pool(name="ppm", bufs=3, space="PSUM") as ppm, \
           tc.tile_pool(name="ppt", bufs=1, space="PSUM") as ppt:

        dma = nc.sync.dma_start
        xT = [pers.tile([128, T], F32, tag=f"xT{d}", name=f"xT{d}") for d in range(8)]
        for d in range(8):
            dma(out=xT[d], in_=P["x0T"][d*128:(d+1)*128, :])
        cosA = pers.tile([128, 32], F32, tag="cosA", name="cosA"); dma(out=cosA, in_=P["cosq"][0])
        cosB = pers.tile([128, 32], F32, tag="cosB", name="cosB"); dma(out=cosB, in_=P["cosq"][1])
        sinA = pers.tile([128, 32], F32, tag="sinA", name="sinA"); dma(out=sinA, in_=P["sinq"][0])
        sinB = pers.tile([128, 32], F32, tag="sinB", name="sinB"); dma(out=sinB, in_=P["sinq"][1])
        mAg = pers.tile([128, 512], F16, tag="mAg", name="mAg"); dma(out=mAg, in_=P["mAg"][:, :])
        mBg = pers.tile([128, 512], F16, tag="mBg", name="mBg"); dma(out=mBg, in_=P["mBg"][:, :])
        mAl = pers.tile([128, 512], F16, tag="mAl", name="mAl"); dma(out=mAl, in_=P["mAl"][:, :])
        mBl = pers.tile([128, 768], F16, tag="mBl", name="mBl"); dma(out=mBl, in_=P["mBl"][:, :])
        idn = pers.tile([128, 128], F16, tag="idn", name="idn")
        make_identity(nc, idn)
        ones16 = pers.tile([128, 1], F16, tag="ones16", name="ones16")
        nc.vector.memset(ones16, 1.0)
        epst = pers.tile([1, 1], F32, tag="epst", name="epst")
        nc.vector.memset(epst, EPS)

        def rmsnorm(nw_dram):
            nw = sm.tile([128, 8], F32, tag="nw", name="nw")
            dma(out=nw, in_=nw_dram)
            ssp = ppm.tile([128, T], F32, tag="pm", name="ssp")
            for d in range(8):
                x2 = act.tile([128, T], F16, tag="x2", name="x2", bufs=2)
                nc.vector.tensor_mul(out=x2, in0=xT[d], in1=xT[d])
                nc.tensor.matmul(ssp[0:1, :], lhsT=ones16, rhs=x2, start=(d == 0), stop=(d == 7))
            lnr = sm.tile([1, T], F32, tag="lnr", name="lnr")
            nc.scalar.activation(out=lnr, in_=ssp[0:1, :], func=AF.Ln, scale=1.0/D,
                                 bias=epst[0:1, 0:1])
            rr = sm.tile([1, T], F32, tag="rr", name="rr")
            nc.scalar.activation(out=rr, in_=lnr, func=AF.Exp, scale=-0.5)
            rb = act.tile([128, T], F32, tag="rb", name="rb")
            nc.gpsimd.partition_broadcast(rb[:], rr[:])
            out = []
            for d in range(8):
                h = act.tile([128, T], F16, tag=f"hT{d}", name=f"hT{d}", bufs=1)
                nc.vector.scalar_tensor_tensor(out=h, in0=xT[d], scalar=nw[:, d:d+1],
                                               in1=rb, op0=ALU.mult, op1=ALU.mult)
                out.append(h)
            return out

        def rope_tok(ps, cost, sint, outt, nheads):
            ev = ps.rearrange("p (h f two) -> p h f two", two=2, f=32)
            ov = outt.rearrange("p (h f two) -> p h f two", two=2, f=32)
            cb = cost[:].rearrange("p (o f) -> p o f", o=1).to_broadcast((128, nheads, 32))
            sb = sint[:].rearrange("p (o f) -> p o f", o=1).to_broadcast((128, nheads, 32))
            t1 = sm.tile([128, nheads, 32], F32, tag="ropet1", name="ropet1", bufs=2)
            t2 = sm.tile([128, nheads, 32], F32, tag="ropet2", name="ropet2", bufs=2)
            nc.vector.tensor_mul(out=t1, in0=ev[:, :, :, 0], in1=cb)
            nc.vector.tensor_mul(out=t2, in0=ev[:, :, :, 1], in1=sb)
            nc.vector.tensor_sub(out=ov[:, :, :, 0], in0=t1, in1=t2)
            nc.vector.tensor_mul(out=t1, in0=ev[:, :, :, 0], in1=sb)
            nc.vector.tensor_mul(out=t2, in0=ev[:, :, :, 1], in1=cb)
            nc.vector.tensor_add(out=ov[:, :, :, 1], in0=t1, in1=t2)

        for l in range(L):
            is_global = ((l + 1) % GEVERY) == 0
            h1 = rmsnorm(P["n1h"][l])
            # ---- QKV projections (shared stationary h1 slices) ----
            wqt = wp.tile([128, 8, 1024], F16, tag="wbig", name="wqt", bufs=2)
            dma(out=wqt, in_=P["wq"][l].rearrange("(dc p) f -> p dc f", p=128))
            wkt = wp.tile([128, 8, 256], F16, tag="wkt", name="wkt")
            wvt = wp.tile([128, 8, 256], F16, tag="wvt", name="wvt")
            dma(out=wkt, in_=P["wk"][l].rearrange("(dc p) f -> p dc f", p=128))
            dma(out=wvt, in_=P["wv"][l].rearrange("(dc p) f -> p dc f", p=128))
            ktok = act.tile([128, 512], F16, tag="ktok", name="ktok")
            vtok = act.tile([128, 512], F16, tag="vtok", name="vtok")
            qtoks = [act.tile([128, 1024], F16, tag=f"qtok{t}", name=f"qtok{t}", bufs=1) for t in range(2)]
            for t2_ in range(2):
                psk = ppm.tile([128, 256], F32, tag="pm", name="psk")
                psv = ppm.tile([128, 256], F32, tag="pm", name="psv")
                psq0 = ppb.tile([128, 512], F32, tag="pb", name="psq0")
                psq1 = ppb.tile([128, 512], F32, tag="pb", name="psq1")
                for d in range(8):
                    lh = h1[d][:, t2_*128:(t2_+1)*128]
                    st, sp = (d == 0), (d == 7)
                    nc.tensor.matmul(psk, lhsT=lh, rhs=wkt[:, d, :], start=st, stop=sp)
                    nc.tensor.matmul(psv, lhsT=lh, rhs=wvt[:, d, :], start=st, stop=sp)
                    nc.tensor.matmul(psq0, lhsT=lh, rhs=wqt[:, d, 0:512], start=st, stop=sp)
                    nc.tensor.matmul(psq1, lhsT=lh, rhs=wqt[:, d, 512:1024], start=st, stop=sp)
                rope_tok(psk, (cosA, cosB)[t2_], (sinA, sinB)[t2_], ktok[:, t2_*256:(t2_+1)*256], 4)
                nc.scalar.activation(out=vtok[:, t2_*256:(t2_+1)*256], in_=psv, func=AF.Copy)
                rope_tok(psq0, (cosA, cosB)[t2_], (sinA, sinB)[t2_], qtoks[t2_][:, 0:512], 8)
                rope_tok(psq1, (cosA, cosB)[t2_], (sinA, sinB)[t2_], qtoks[t2_][:, 512:1024], 8)
            # ---- transpose k (4 blocks) into kv_in layout ----
            kTsb = act.tile([128, 512], F16, tag="kTsb", name="kTsb")
            pstk = ppt.tile([128, 1024], F16, tag="pstr", name="pstk")
            for i in range(2):
                for t2_ in range(2):
                    nc.tensor.transpose(pstk[:, i*256 + t2_*128:i*256 + (t2_+1)*128],
                                        ktok[:, t2_*256 + i*128:t2_*256 + (i+1)*128], idn)
            nc.scalar.activation(out=kTsb, in_=pstk[:, 0:512], func=AF.Copy)
            for i in range(2):
                dma(out=kv_in[l][i], in_=kTsb[:, i*256:(i+1)*256])
                dma(out=kv_in[l][2 + i], in_=vtok[:, i*256:(i+1)*256])
            if SMALL_COLL:
                dma(out=kv_in_s[l][0, :, :], in_=kTsb[0:1, 0:256])
                nc.gpsimd.collective_compute(
                    "AllGather", mybir.AluOpType.bypass, replica_groups=RG_KV,
                    ins=[kv_in_s[l].ap()], outs=[kv_out_s[l].ap()])

            elif not SKIP_COLL:
                nc.gpsimd.collective_compute(
                    "AllGather", mybir.AluOpType.bypass, replica_groups=RG_KV,
                    ins=[kv_in[l].ap()], outs=[kv_out[l].ap()])
            # ---- transpose q (16 blocks) -> qT [128, 8, 256] ----
            qT = atp.tile([128, 8, 256], F16, tag="qT", name="qT")
            for t2_ in range(2):
                pst = ppt.tile([128, 1024], F16, tag="pstr", name="pst")
                for f in range(8):
                    nc.tensor.transpose(pst[:, f*128:(f+1)*128],
                                        qtoks[t2_][:, f*128:(f+1)*128], idn)
                dstv = qT[:, :, t2_*128:(t2_+1)*128]
                srcv = pst.rearrange("p (f c) -> p f c", f=8)
                if t2_ == 0:
                    nc.vector.tensor_copy(out=dstv, in_=srcv)
                else:
                    nc.scalar.activation(out=dstv, in_=srcv, func=AF.Copy)
            # ---- gather K/V from collective ----
            kT_full = [atp.tile([128, 1024], F16, tag=f"kTf{i}", name=f"kTf{i}", bufs=1) for i in range(2)]
            kvo = kv_out[l].rearrange("(r x) p c -> x r p c", x=4)
            for i in range(2):
                dma(out=kT_full[i].rearrange("p (sl r c) -> p sl r c", sl=2, r=4),
                    in_=kvo[i].rearrange("r p (sl c) -> p sl r c", sl=2))
            if SMALL_COLL:
                dma(out=kT_full[0][0:1, 0:256], in_=kv_out_s[l][0])
            v_full = atp.tile([128, 8, 4, 64], F16, tag="vfull", name="vfull")
            for sl in range(2):
                dma(out=v_full[:, sl*4:(sl+1)*4, :, :],
                    in_=kvo[2 + sl].rearrange("r p (h f) -> p r h f", f=64))
            # ---- attention ----
            oT = atp.tile([128, 8, 256], F16, tag="oT", name="oT")
            for qb in range(2):
                if qb == 0:
                    chunks = [(0, 512, mAg if is_global else mAl)]
                    kblocks = [0, 1, 2, 3]
                elif is_global:
                    chunks = [(0, 512, None), (512, 512, mBg)]
                    kblocks = list(range(8))
                else:
                    chunks = [(256, 512, mBl[:, 0:512]), (768, 256, mBl[:, 512:768])]
                    kblocks = [2, 3, 4, 5, 6, 7]
                kn = sum(w for _, w, _ in chunks)
                nblk = len(kblocks)
                ops = [ppo.tile([128, 512], F32, tag=f"ops{t}", name=f"ops{t}", bufs=1) for t in range(2)]
                for kvh in range(4 if not SKIP_ATTN else 0):
                    ktile = kT_full[kvh // 2]
                    for half in range(2):
                        slots = KVGROUPS[kvh][half*2:(half+1)*2]
                        attTs = []
                        for si, g in enumerate(slots):
                            base = (g % 2) * 64
                            lhq = qT[base:base+64, g//2, qb*128:(qb+1)*128]
                            probs = atp.tile([128, 1024], F16, tag=f"probs{si}", name=f"probs{si}", bufs=1)
                            spss = []
                            for (c0, w, msk) in chunks:
                                sps = ppb.tile([128, 512], F32, tag="pb", name="sps")
                                nc.tensor.matmul(sps[:, 0:w], lhsT=lhq,
                                                 rhs=ktile[base:base+64, c0:c0+w],
                                                 start=True, stop=(msk is None))
                                spss.append(sps)
                            for sps, (c0, w, msk) in zip(spss, chunks):
                                if msk is not None:
                                    nc.tensor.matmul(sps[:, 0:w], lhsT=idn, rhs=msk,
                                                     start=False, stop=True)
                            off = 0
                            accs = []
                            for sps, (c0, w, msk) in zip(spss, chunks):
                                acc = sm.tile([128, 1], F32, tag="acc", name="acc")
                                nc.scalar.activation(out=probs[:, off:off+w], in_=sps[:, 0:w],
                                                     func=AF.Exp, scale=SCALE, accum_out=acc)
                                accs.append(acc)
                                off += w
                            if len(accs) == 2:
                                nc.vector.tensor_add(out=accs[0], in0=accs[0], in1=accs[1])
                            rec = sm.tile([128, 1], F32, tag="rec", name="rec")
                            nc.vector.reciprocal(out=rec, in_=accs[0])
                            nc.vector.tensor_scalar_mul(out=probs[:, 0:kn], in0=probs[:, 0:kn], scalar1=rec)
                            attT = atp.tile([128, 1024], F16, tag=f"attT{si}", name=f"attT{si}", bufs=1)
                            pstr = ppt.tile([128, 1024], F16, tag="pstr", name="pstr")
                            for j in range(nblk):
                                nc.tensor.transpose(pstr[:, j*128:(j+1)*128],
                                                    probs[:, j*128:(j+1)*128], idn)
                            if si == 0:
                                nc.vector.tensor_copy(out=attT[:, 0:nblk*128],
                                                      in_=pstr[:, 0:nblk*128])
                            else:
                                nc.scalar.activation(out=attT[:, 0:nblk*128],
                                                     in_=pstr[:, 0:nblk*128], func=AF.Copy)
                            attTs.append(attT)
                        for si, g in enumerate(slots):
                            for bi in range(nblk):
                                b = kblocks[bi]
                                nc.tensor.matmul(ops[g // 8][(g % 2)*64:(g % 2)*64 + 64,
                                                             ((g // 2) % 4)*128:((g // 2) % 4 + 1)*128],
                                                 lhsT=v_full[:, b, kvh, :],
                                                 rhs=attTs[si][:, bi*128:(bi+1)*128],
                                                 start=(bi == 0), stop=(bi == nblk - 1),
                                                 tile_position=(0, (g % 2)*64))
                for t_ in range(2):
                    dstv = oT[:, t_*4:(t_+1)*4, qb*128:(qb+1)*128]
                    srcv = ops[t_].rearrange("p (f c) -> p f c", f=4)
                    if t_ == 0:
                        nc.vector.tensor_copy(out=dstv, in_=srcv)
                    else:
                        nc.scalar.activation(out=dstv, in_=srcv, func=AF.Copy)
            # ---- O projection ----
            wot = wp.tile([128, 8, 1024], F16, tag="wbig", name="wot", bufs=2)
            dma(out=wot, in_=P["wo"][l].rearrange("(ft p) c -> p ft c", p=128))
            for d in range(8):
                pso = ppm.tile([128, 256], F32, tag="pm", name="pso")
                for ft in range(8):
                    nc.tensor.matmul(pso, lhsT=wot[:, ft, d*128:(d+1)*128], rhs=oT[:, ft, :],
                                     start=(ft == 0), stop=(ft == 7))
                nc.vector.tensor_add(out=xT[d], in0=xT[d], in1=pso)
            # ---- FFN ----
            h2 = rmsnorm(P["n2h"][l])
            yT = []
            ci = 0
            for (c0, cw) in W13CH:
                w1t = wp.tile([128, 8, 512], F16, tag="w1c", name="w1c", bufs=2)
                w3t = wp.tile([128, 8, 512], F16, tag="w3c", name="w3c", bufs=2)
                dma(out=w1t[:, :, 0:cw], in_=P["w1"][l, :, c0:c0+cw].rearrange("(dc p) h -> p dc h", p=128))
                dma(out=w3t[:, :, 0:cw], in_=P["w3"][l, :, c0:c0+cw].rearrange("(dc p) h -> p dc h", p=128))
                for hc0 in range(0, cw, 128):
                    hw = min(128, cw - hc0)
                    psu = ppm.tile([128, 256], F32, tag="pm", name="psu")
                    psg = ppm.tile([128, 256], F32, tag="pm", name="psg")
                    for d in range(8):
                        nc.tensor.matmul(psu[0:hw, :], lhsT=w1t[:, d, hc0:hc0+hw], rhs=h2[d],
                                         start=(d == 0), stop=(d == 7))
                    for d in range(8):
                        nc.tensor.matmul(psg[0:hw, :], lhsT=w3t[:, d, hc0:hc0+hw], rhs=h2[d],
                                         start=(d == 0), stop=(d == 7))
                    su = act.tile([128, 256], F32, tag="su", name="su", bufs=1)
                    if SIM_SILU:
                        nc.scalar.activation(out=su[0:hw, :], in_=psu[0:hw, :], func=AF.Sigmoid)
                        nc.vector.tensor_mul(out=su[0:hw, :], in0=su[0:hw, :], in1=psu[0:hw, :])
                    else:
                        nc.scalar.activation(out=su[0:hw, :], in_=psu[0:hw, :], func=AF.Silu)
                    y = act.tile([128, 256], F16, tag=f"yT{ci}", name=f"yT{ci}", bufs=1)
                    nc.vector.tensor_mul(out=y[0:hw, :], in0=su[0:hw, :], in1=psg[0:hw, :])
                    yT.append(y)
                    ci += 1
            w2a = wp.tile([128, 11, 1024], F16, tag="w2a", name="w2a")
            w2b = wp.tile([128, 10, 1024], F16, tag="w2b", name="w2b")
            w2x = wp.tile([64, 1024], F16, tag="w2x", name="w2x")
            dma(out=w2a, in_=P["w2"][l, 0:1408, :].rearrange("(hc p) c -> p hc c", p=128))
            dma(out=w2b, in_=P["w2"][l, 1408:2688, :].rearrange("(hc p) c -> p hc c", p=128))
            dma(out=w2x, in_=P["w2"][l, 2688:2752, :])
            for d in range(8):
                ps2 = ppm.tile([128, 256], F32, tag="pm", name="ps2")
                for ci2, (h0, hwid) in enumerate(HCHUNKS):
                    if hwid == 64:
                        lh = w2x[:, d*128:(d+1)*128]
                    elif h0 < 1408:
                        lh = w2a[:, h0 // 128, d*128:(d+1)*128]
                    else:
                        lh = w2b[:, (h0 - 1408) // 128, d*128:(d+1)*128]
                    nc.tensor.matmul(ps2, lhsT=lh, rhs=yT[ci2][0:hwid, :],
                                     start=(ci2 == 0), stop=(ci2 == len(HCHUNKS) - 1))
                nc.vector.tensor_add(out=xT[d], in0=xT[d], in1=ps2)
        # ---- final norm + tied lm head ----
        xf = rmsnorm(P["nfh"][:, :])
        for d in range(8):
            dma(out=xf_in[d*128:(d+1)*128, :], in_=xf[d])
        if SMALL_COLL:
            dma(out=xf_in_s[:, :], in_=xf[0][0:8, :])
            nc.gpsimd.collective_compute(
                "AllGather", mybir.AluOpType.bypass, replica_groups=RG_ALL,
                ins=[xf_in_s.ap()], outs=[xf_out_s.ap()])
        elif not SKIP_COLL:
            nc.gpsimd.collective_compute(
                "AllGather", mybir.AluOpType.bypass, replica_groups=RG_ALL,
                ins=[xf_in.ap()], outs=[xf_out.ap()])
        xfv = xf_out.rearrange("(r dc p) t -> p dc r t", p=128, dc=8)
        VCH = [(i * 256, 256) for i in range(15)] + [(3840, 160)]
        for th in range(2):
            xft = wp.tile([128, 8, 1024], F16, tag="wbig", name="xft", bufs=2)
            for d in range(8):
                dma(out=xft[:, d, :].rearrange("p (r t) -> p r t", r=4),
                    in_=xfv[:, d, th*4:(th+1)*4, :])
            if SMALL_COLL:
                dma(out=xft[0:1, 0, 0:64], in_=xf_out_s[0, 0:64])
            for vci, (v0, vw) in enumerate(VCH):
                embt = wp.tile([128, 8, 256], F16, tag=("wkt", "wvt")[vci % 2], name="embt")
                dma(out=embt[:, :, 0:vw],
                    in_=P["embT"][:, v0:v0+vw].rearrange("(dc p) v -> p dc v", p=128))
                for vb0 in range(0, vw, 128) if not SKIP_LM else []:
                    vbw = min(128, vw - vb0)
                    psl = [ppb.tile([128, 512], F32, tag="pb", name="psl") for _ in range(2)]
                    for d in range(8):
                        for tb in range(2):
                            nc.tensor.matmul(psl[tb][0:vbw, :], lhsT=embt[:, d, vb0:vb0+vbw],
                                             rhs=xft[:, d, tb*512:(tb+1)*512],
                                             start=(d == 0), stop=(d == 7))
                    lg = act.tile([128, 1024], F16, tag=f"qtok{(vb0 // 128) % 2}", name="lg")
                    nc.vector.tensor_copy(out=lg[0:vbw, 0:512], in_=psl[0][0:vbw, :])
                    nc.scalar.activation(out=lg[0:vbw, 512:1024], in_=psl[1][0:vbw, :],
                                         func=AF.Copy)
                    dma(out=logits[v0+vb0:v0+vb0+vbw, th*1024:(th+1)*1024],
                        in_=lg[0:vbw, :])
    nc.compile()
    return nc

_NC_CACHE = {}
def _get_nc():
    if "nc" not in _NC_CACHE:
        _NC_CACHE["nc"] = _build_nc()
    return _NC_CACHE["nc"]

def kernel(**inputs):
    from concourse.bass_utils import run_bass_kernel_spmd
    nc = _get_nc()
    in_maps = _host_prep(**inputs)
    res = run_bass_kernel_spmd(nc, in_maps, list(range(NCORES)))
    return _assemble(res.results)
